# revision 19
# baseline (speedup 1.0000x reference)
"""Trainium2 Bass kernel for nn_Decoder (4-block decoder with scatter-memory
read/write).  Data-parallel over batch: 8 batches per NeuronCore x 8 cores.

Per-core activation layout ("x2"): [128, 2048] f32 where
  row = 64*g + d      (g = local_batch // 4, d = model dim 0..64)
  col = 512*j + s     (j = local_batch % 4,  s = seq position 0..512)

All weights are packed host-side (padded head layouts, per-partition-group
duplicates) so the device kernel is pure matmul/elementwise work.
"""
import os
import sys

sys.path.insert(0, "/opt/trn_rl_repo")

import numpy as np

import concourse.bacc as bacc
import concourse.mybir as mybir
import concourse.tile as tile
from concourse import bass_utils

F32 = mybir.dt.float32
AF = mybir.ActivationFunctionType
ALU = mybir.AluOpType

B, S, D = 64, 512, 64
H, HD = 4, 16
A, M, K = 32, 64, 8
HID = 256
NB = 4
RT, WT = 0.25, 0.25
EPS = 1e-5
NCORES = 8
BL = B // NCORES          # local batches per core = 8
NTOK = BL * S             # 4096 local tokens
FREE = NTOK // 2          # 2048 free size of x2
NCH = NTOK // 128         # 32 token chunks of 128

_PAD_COLS = (np.arange(64) // 16) * 32 + np.arange(64) % 16  # head-pad map
# compact exp_w windows per kc: sizes 512,384,256,128 (x4 heads)
_EXPOFF = [0, 2048, 3584, 4608]
_EXPTOT = 5120


def _build(gelu_sim=False):
    nc = bacc.Bacc("TRN2", target_bir_lowering=False, debug=False,
                   enable_asserts=True)
    dram = {}

    def din(name, shape):
        dram[name] = nc.dram_tensor(name, shape, F32, kind="ExternalInput")

    def dout(name, shape):
        dram[name] = nc.dram_tensor(name, shape, F32, kind="ExternalOutput")

    din("x2", (128, FREE))
    din("wq", (128, NB, 128)); din("wk", (128, NB, 128))
    din("wv", (128, NB, 64));  din("wo", (128, NB, 64))
    din("w1", (128, NB, 256)); din("b1", (128, NB, 2))
    din("w2", (128, NB, 2, 64)); din("b2row", (1, NB, 64))
    din("readq", (128, NB, 32)); din("readout", (64, NB, 64))
    din("ew", (2, 14 * 128))
    din("e2", (128, 2)); din("e2a", (64, 2)); din("e2at", (2, 64))
    din("ehw", (128, 128)); din("negi", (128, 128)); din("triu", (128, 128))
    din("ident", (128, 128)); din("addrn", (64, 64))
    din("memaug", (64, BL, 65)); din("memold", (64, BL, 64))
    din("wqw", (128, 32)); din("wvw", (128, 64)); din("gw", (128, 1))
    din("gbneg", (128, 1))
    dout("xout2", (128, FREE))
    dout("memout", (64, BL * 64))

    import contextlib
    with tile.TileContext(nc) as tc, contextlib.ExitStack() as ctx:
        _emit(nc, tc, dram, ctx, gelu_sim)
    nc.compile()
    return nc


def _emit(nc, tc, dram, ctx, gelu_sim):
    wpool = ctx.enter_context(tc.tile_pool(name="w", bufs=1))
    xpool = ctx.enter_context(tc.tile_pool(name="x", bufs=2))
    tpool = ctx.enter_context(tc.tile_pool(name="t", bufs=1))
    t2pool = ctx.enter_context(tc.tile_pool(name="t2", bufs=2))
    spool = ctx.enter_context(tc.tile_pool(name="s", bufs=1))
    s2pool = ctx.enter_context(tc.tile_pool(name="s2", bufs=1))
    ppool = ctx.enter_context(tc.tile_pool(name="ps4", bufs=1, space="PSUM"))
    qpool = ctx.enter_context(tc.tile_pool(name="ps1", bufs=4, space="PSUM"))

    # ---- persistent weights/constants -----------------------------------
    W = {}
    for name in ["wq", "wk", "wv", "wo", "w1", "b1", "w2", "b2row", "readq",
                 "readout", "ew", "e2", "e2a", "e2at", "ehw", "negi", "triu",
                 "ident", "addrn", "memaug", "memold", "wqw", "wvw", "gw",
                 "gbneg"]:
        t = wpool.tile(list(dram[name].shape), F32, tag=f"w_{name}")
        nc.sync.dma_start(t[:], dram[name].ap())
        W[name] = t

    ones_row = wpool.tile([1, 512], F32, tag="w_ones_row")
    nc.vector.memset(ones_row[:], 1.0)
    ones1_64 = wpool.tile([1, 64], F32, tag="w_ones64")
    nc.vector.memset(ones1_64[:], 1.0)
    epsb2 = wpool.tile([2, 1], F32, tag="w_epsb2")
    nc.vector.memset(epsb2[:], EPS)
    tinyb2 = wpool.tile([2, 1], F32, tag="w_tinyb2")
    nc.vector.memset(tinyb2[:], 1e-30)

    x_cur = xpool.tile([128, FREE], F32, tag="x")
    nc.sync.dma_start(x_cur[:], dram["x2"].ap())

    # ---- rms norm: x * rsqrt(mean_d(x^2)+eps) * w -----------------------
    def rms(x_t, norm_idx, out_pool=None, out_tag="xn"):
        sq = tpool.tile([128, FREE], F32, tag="sq")
        nc.vector.tensor_tensor(out=sq[:], in0=x_t[:], in1=x_t[:], op=ALU.mult)
        msp = ppool.tile([128, FREE], F32, tag="ps4")
        for c in range(4):
            cs = slice(512 * c, 512 * c + 512)
            nc.tensor.matmul(msp[:2, cs], W["e2"][:], sq[:, cs],
                             start=True, stop=True)
        nc.scalar.activation(msp[32:34, :], msp[:2, :], AF.Ln, bias=epsb2[:],
                             scale=1.0 / 64.0)
        rs2 = spool.tile([2, FREE], F32, tag="rs2")
        nc.scalar.activation(rs2[:], msp[32:34, :], AF.Exp, bias=0.0,
                             scale=-0.5)
        rsp = ppool.tile([128, FREE], F32, tag="ps4")
        ewsl = W["ew"][:, 128 * norm_idx:128 * norm_idx + 128]
        for c in range(4):
            cs = slice(512 * c, 512 * c + 512)
            nc.tensor.matmul(rsp[:, cs], ewsl, rs2[:, cs],
                             start=True, stop=True)
        xn = (out_pool or tpool).tile([128, FREE], F32, tag=out_tag)
        nc.vector.tensor_tensor(out=xn[:], in0=x_t[:], in1=rsp[:], op=ALU.mult)
        return xn

    # ---- per-token query scale 1/(||q||*temp), rows 32g+a ---------------
    def qscale_pad(q_sb, temp):
        sqr = tpool.tile([128, FREE], F32, tag="sq")
        nc.vector.tensor_tensor(out=sqr[:64, :], in0=q_sb[:], in1=q_sb[:],
                                op=ALU.mult)
        qqp = ppool.tile([128, FREE], F32, tag="ps4")
        for c in range(4):
            cs = slice(512 * c, 512 * c + 512)
            nc.tensor.matmul(qqp[:2, cs], W["e2a"][:], sqr[:64, cs],
                             start=True, stop=True)
        nc.scalar.activation(qqp[32:34, :], qqp[:2, :], AF.Ln, bias=tinyb2[:],
                             scale=temp * temp)
        rq2 = spool.tile([2, FREE], F32, tag="rs2")
        nc.scalar.activation(rq2[:], qqp[32:34, :], AF.Exp, bias=0.0,
                             scale=-0.5)
        rqp = ppool.tile([128, FREE], F32, tag="ps4")
        for c in range(4):
            cs = slice(512 * c, 512 * c + 512)
            nc.tensor.matmul(rqp[:64, cs], W["e2at"][:], rq2[:, cs],
                             start=True, stop=True)
        return rqp

    # ---- 32-chunk address scores [128, 64] per token chunk --------------
    def addr_scores(q2s):
        scp = ppool.tile([128, FREE], F32, tag="ps4")
        for c in range(NCH):
            g = c // 16
            lh = q2s[32 * g:32 * g + 32, 128 * (c % 16):128 * (c % 16) + 128]
            nc.tensor.matmul(scp[:, 64 * c:64 * c + 64], lh,
                             W["addrn"][32 * g:32 * g + 32, :],
                             start=(c % 8 == 0), stop=(c % 8 == 7),
                             tile_position=(32 * g, 0), skip_group_check=True)
        return scp

    # =====================================================================
    for i in range(NB):
        # ---------------- attention ----------------
        xn = rms(x_cur, 3 * i + 0)
        qkv = {}
        for g in range(2):
            gr = slice(64 * g, 64 * g + 64)
            for nm, wt in [("q", "wq"), ("k", "wk")]:
                pp = ppool.tile([128, FREE], F32, tag="ps4")
                for c in range(4):
                    cs = slice(512 * c, 512 * c + 512)
                    nc.tensor.matmul(pp[:, cs], W[wt][gr, i, :], xn[gr, cs],
                                     start=True, stop=True,
                                     tile_position=(64 * g, 0))
                sb = t2pool.tile([128, FREE], F32, tag=f"{nm}sb")
                nc.vector.tensor_copy(sb[:], pp[:])
                qkv[(nm, g)] = sb
            vp = ppool.tile([128, FREE], F32, tag="ps4")
            for c in range(16):
                nc.tensor.matmul(vp[:, 64 * c:64 * c + 64],
                                 xn[gr, 128 * c:128 * c + 128],
                                 W["wv"][gr, i, :],
                                 start=(c % 8 == 0), stop=(c % 8 == 7),
                                 tile_position=(64 * g, 0),
                                 skip_group_check=True)
            vsb = t2pool.tile([128, 16, 4, 32], F32, tag="vsb")
            for j in range(4):
                nc.vector.tensor_copy(
                    vsb[:, 4 * j:4 * j + 4, :, 0:16],
                    vp[:, 256 * j:256 * j + 256].rearrange(
                        "p (kc h hd) -> p kc h hd", kc=4, h=4))
            nc.vector.memset(vsb[:, :, :, 16:32], 1.0)
            qkv[("v", g)] = vsb

        x_att = xpool.tile([128, FREE], F32, tag="x")
        for b in range(BL):
            g, j = b // 4, b % 4
            qg, kg, vg = qkv[("q", g)], qkv[("k", g)], qkv[("v", g)]
            expw = spool.tile([128, _EXPTOT], F32, tag="expw")
            for kc in range(4):
                n = 512 - 128 * kc
                scp = ppool.tile([128, FREE], F32, tag="ps4")
                for h in range(4):
                    hr = slice(32 * h, 32 * h + 32)
                    ks = slice(512 * j + 128 * kc, 512 * j + 128 * kc + 128)
                    qs = slice(512 * j + 128 * kc, 512 * j + 512)
                    nc.tensor.matmul(scp[:, 512 * h:512 * h + n],
                                     kg[hr, ks], qg[hr, qs],
                                     start=True, stop=False,
                                     tile_position=(32 * h, 0),
                                     skip_group_check=True)
                    nc.tensor.matmul(scp[:, 512 * h:512 * h + 128],
                                     W["negi"][:], W["triu"][:],
                                     start=False, stop=True,
                                     skip_group_check=True)
                eap = expw[:, _EXPOFF[kc]:_EXPOFF[kc] + 4 * n].rearrange(
                    "p (h q) -> p h q", h=4)
                nc.scalar.activation(
                    eap,
                    scp[:, :].rearrange("p (h q) -> p h q", h=4)[:, :, 0:n],
                    AF.Exp)
            avp = qpool.tile([128, 512], F32, tag="ps1")
            for kc in range(4):
                n = 512 - 128 * kc
                for h in range(4):
                    nc.tensor.matmul(
                        avp[32 * h:32 * h + 32, 128 * kc:512],
                        vg[:, 4 * j + kc, h, :],
                        expw[:, _EXPOFF[kc] + h * n:_EXPOFF[kc] + h * n + n],
                        start=(kc == 0), stop=(kc == 3),
                        tile_position=(0, 32 * h), skip_group_check=True)
            rca = s2pool.tile([128, 512], F32, tag="rca")
            nc.vector.reciprocal(out=rca[:], in_=avp[:])
            rzp = qpool.tile([128, 512], F32, tag="ps1")
            nc.tensor.matmul(rzp[:], W["ehw"][:], rca[:], start=True,
                             stop=True)
            asb = s2pool.tile([128, 512], F32, tag="asb")
            nc.vector.tensor_copy(asb[:], avp[:])
            ann = s2pool.tile([128, 512], F32, tag="ann")
            nc.vector.tensor_tensor(out=ann[:], in0=asb[:], in1=rzp[:],
                                    op=ALU.mult)
            op = qpool.tile([128, 512], F32, tag="ps1")
            gr = slice(64 * g, 64 * g + 64)
            nc.tensor.matmul(op[gr, :], W["wo"][:, i, :], ann[:],
                             start=True, stop=True,
                             tile_position=(0, 64 * g),
                             skip_group_check=True)
            js = slice(512 * j, 512 * j + 512)
            nc.vector.tensor_tensor(out=x_att[gr, js], in0=op[gr, :],
                                    in1=x_cur[gr, js], op=ALU.add)

        # ---------------- memory read ----------------
        xnr = rms(x_att, 3 * i + 1)
        qrp = ppool.tile([128, FREE], F32, tag="ps4")
        for g in range(2):
            gr = slice(64 * g, 64 * g + 64)
            for c in range(4):
                cs = slice(512 * c, 512 * c + 512)
                nc.tensor.matmul(qrp[32 * g:32 * g + 32, cs],
                                 W["readq"][gr, i, :], xnr[gr, cs],
                                 start=True, stop=True,
                                 tile_position=(64 * g, 32 * g),
                                 skip_group_check=True)
        qrs = tpool.tile([64, FREE], F32, tag="qrs")
        nc.vector.tensor_copy(qrs[:], qrp[:64, :])
        rqp = qscale_pad(qrs, RT)
        q2s = tpool.tile([64, FREE], F32, tag="q2s")
        nc.vector.tensor_tensor(out=q2s[:], in0=qrs[:], in1=rqp[:64, :],
                                op=ALU.mult)
        scp = addr_scores(q2s)
        scs = tpool.tile([128, FREE], F32, tag="scs")
        nc.vector.tensor_copy(scs[:], scp[:])

        msk = tpool.tile([128, FREE], F32, tag="sq")
        for c in range(NCH):
            m8 = spool.tile([128, 8], F32, tag="m8")
            nc.vector.max(out=m8[:], in_=scs[:, 64 * c:64 * c + 64])
            nc.vector.tensor_tensor(
                out=msk[:, 64 * c:64 * c + 64],
                in0=scs[:, 64 * c:64 * c + 64],
                in1=m8[:, 7:8].to_broadcast((128, 64)),
                op=ALU.is_ge)
        exr = t2pool.tile([128, FREE], F32, tag="qsb")
        nc.scalar.activation(exr[:], scs[:], AF.Exp)
        wu = t2pool.tile([128, FREE], F32, tag="ksb")
        nc.vector.tensor_tensor(out=wu[:], in0=exr[:], in1=msk[:],
                                op=ALU.mult)
        x_read = xpool.tile([128, FREE], F32, tag="x")
        for half in range(2):
            wtp = ppool.tile([128, FREE], F32, tag="ps4")
            for cc in range(16):
                c = 16 * half + cc
                nc.tensor.matmul(wtp[:64, 128 * cc:128 * cc + 128],
                                 wu[:, 64 * c:64 * c + 64], W["ident"][:],
                                 is_transpose=True,
                                 start=(cc % 4 == 0), stop=(cc % 4 == 3),
                                 skip_group_check=True)
            wts = t2pool.tile([64, FREE], F32, tag="vsb")
            nc.vector.tensor_copy(wts[:], wtp[:64, :])
            for b in range(4 * half, 4 * half + 4):
                g, j = b // 4, b % 4
                rvp = qpool.tile([128, 512], F32, tag="ps1")
                nc.tensor.matmul(rvp[:65, :], W["memaug"][:, b, :],
                                 wts[:, 512 * (b - 4 * half):
                                     512 * (b - 4 * half) + 512],
                                 start=True, stop=True)
                rdn = s2pool.tile([1, 512], F32, tag="rdn")
                nc.vector.reciprocal(out=rdn[:], in_=rvp[64:65, :])
                rdp = qpool.tile([128, 512], F32, tag="ps1")
                nc.tensor.matmul(rdp[:64, :], ones1_64[:], rdn[:],
                                 start=True, stop=True)
                rvs = s2pool.tile([64, 512], F32, tag="rvs")
                nc.vector.tensor_copy(rvs[:], rvp[:64, :])
                rvn = s2pool.tile([64, 512], F32, tag="rvn")
                nc.vector.tensor_tensor(out=rvn[:], in0=rvs[:],
                                        in1=rdp[:64, :], op=ALU.mult)
                xrp = qpool.tile([128, 512], F32, tag="ps1")
                gr = slice(64 * g, 64 * g + 64)
                nc.tensor.matmul(xrp[gr, :], W["readout"][:, i, :], rvn[:],
                                 start=True, stop=True,
                                 tile_position=(0, 64 * g),
                                 skip_group_check=True)
                js = slice(512 * j, 512 * j + 512)
                nc.vector.tensor_tensor(out=x_read[gr, js], in0=xrp[gr, :],
                                        in1=x_att[gr, js], op=ALU.add)

        # ---------------- ffn ----------------
        xnf = rms(x_read, 3 * i + 2)
        x_ffn = xpool.tile([128, FREE], F32, tag="x")
        for g in range(2):
            gr = slice(64 * g, 64 * g + 64)
            hgs = []
            for co in range(2):
                hp = ppool.tile([128, FREE], F32, tag="ps4")
                for c in range(4):
                    cs = slice(512 * c, 512 * c + 512)
                    nc.tensor.matmul(hp[:, cs],
                                     W["w1"][gr, i, 128 * co:128 * co + 128],
                                     xnf[gr, cs], start=True, stop=True,
                                     tile_position=(64 * g, 0))
                hg = t2pool.tile([128, FREE], F32, tag="h1g")
                if gelu_sim:
                    hb = tpool.tile([128, FREE], F32, tag="sq")
                    nc.scalar.activation(hb[:], hp[:], AF.Identity,
                                         bias=W["b1"][:, i, co:co + 1],
                                         scale=1.0)
                    sgm = tpool.tile([128, FREE], F32, tag="qrs")
                    nc.scalar.activation(sgm[:], hb[:], AF.Sigmoid,
                                         bias=0.0, scale=1.702)
                    nc.vector.tensor_tensor(out=hg[:], in0=hb[:], in1=sgm[:],
                                            op=ALU.mult)
                else:
                    nc.scalar.activation(hg[:], hp[:], AF.Gelu_apprx_tanh,
                                         bias=W["b1"][:, i, co:co + 1],
                                         scale=1.0)
                hgs.append(hg)
            for c in range(4):
                cs = slice(512 * c, 512 * c + 512)
                h2t = qpool.tile([128, 512], F32, tag="ps1")
                for co in range(2):
                    nc.tensor.matmul(h2t[gr, :], W["w2"][:, i, co, :],
                                     hgs[co][:, cs],
                                     start=(co == 0), stop=False,
                                     tile_position=(0, 64 * g),
                                     skip_group_check=True)
                nc.tensor.matmul(h2t[gr, :], W["b2row"][:, i, :],
                                 ones_row[:], start=False, stop=True,
                                 tile_position=(0, 64 * g),
                                 skip_group_check=True)
                nc.vector.tensor_tensor(out=x_ffn[gr, cs], in0=h2t[gr, :],
                                        in1=x_read[gr, cs], op=ALU.add)
        x_cur = x_ffn

    # =============== final norms + memory write ===============
    x_f = rms(x_cur, 12, out_pool=xpool, out_tag="x")
    nc.sync.dma_start(dram["xout2"].ap(), x_f[:])
    xw = rms(x_f, 13, out_pool=xpool, out_tag="x")

    qwp = ppool.tile([128, FREE], F32, tag="ps4")
    for g in range(2):
        gr = slice(64 * g, 64 * g + 64)
        for c in range(4):
            cs = slice(512 * c, 512 * c + 512)
            nc.tensor.matmul(qwp[32 * g:32 * g + 32, cs], W["wqw"][gr, :],
                             xw[gr, cs], start=True, stop=True,
                             tile_position=(64 * g, 32 * g),
                             skip_group_check=True)
    qws = tpool.tile([64, FREE], F32, tag="qrs")
    nc.vector.tensor_copy(qws[:], qwp[:64, :])
    rqwp = qscale_pad(qws, WT)
    qw2s = tpool.tile([64, FREE], F32, tag="q2s")
    nc.vector.tensor_tensor(out=qw2s[:], in0=qws[:], in1=rqwp[:64, :],
                            op=ALU.mult)
    swp = addr_scores(qw2s)
    exw = tpool.tile([128, FREE], F32, tag="scs")
    nc.scalar.activation(exw[:], swp[:], AF.Exp)

    # gates: z = xw @ gw, one bank per g
    zps = []
    for g in range(2):
        gr = slice(64 * g, 64 * g + 64)
        zp = qpool.tile([128, 512], F32, tag="ps1")
        for c in range(16):
            nc.tensor.matmul(zp[:, c:c + 1],
                             xw[gr, 128 * c:128 * c + 128], W["gw"][gr, :],
                             start=(c == 0), stop=(c == 15),
                             tile_position=(64 * g, 0),
                             skip_group_check=True)
        gex = spool.tile([128, 16], F32, tag=f"gex{g}")
        nc.scalar.activation(gex[:], zp[:, 0:16], AF.Exp,
                             bias=W["gbneg"][:], scale=-1.0)
        zps.append(gex)
    gp1 = spool.tile([128, 2, 16], F32, tag="gp1")
    for g in range(2):
        nc.vector.tensor_scalar_add(gp1[:, g, :], zps[g][:], 1.0)
    gat = spool.tile([128, 2, 16], F32, tag="gat")
    nc.vector.reciprocal(out=gat[:], in_=gp1[:])

    den = spool.tile([128, NCH], F32, tag="den")
    nc.vector.tensor_reduce(out=den[:],
                            in_=exw[:].rearrange("p (c m) -> p c m", m=64),
                            axis=mybir.AxisListType.X, op=ALU.add)
    rde = spool.tile([128, NCH], F32, tag="rde")
    nc.vector.reciprocal(out=rde[:], in_=den[:])
    gd = spool.tile([128, NCH, 1], F32, tag="gd")
    nc.vector.tensor_tensor(out=gd[:, :, 0],
                            in0=gat[:].rearrange("p g c -> p (g c)"),
                            in1=rde[:], op=ALU.mult)
    wsb = t2pool.tile([128, NCH, 64], F32, tag="ksb")
    nc.vector.tensor_tensor(out=wsb[:],
                            in0=exw[:].rearrange("p (c m) -> p c m", m=64),
                            in1=gd[:].to_broadcast((128, NCH, 64)),
                            op=ALU.mult)
    # vu = xw @ write_v, augmented with ones column
    vaug = tpool.tile([128, NCH, 65], F32, tag="vaug")
    for g in range(2):
        gr = slice(64 * g, 64 * g + 64)
        vp = ppool.tile([128, FREE], F32, tag="ps4")
        for c in range(16):
            nc.tensor.matmul(vp[:, 64 * c:64 * c + 64],
                             xw[gr, 128 * c:128 * c + 128], W["wvw"][gr, :],
                             start=(c % 8 == 0), stop=(c % 8 == 7),
                             tile_position=(64 * g, 0), skip_group_check=True)
        nc.vector.tensor_copy(
            vaug[:, 16 * g:16 * g + 16, 0:64],
            vp[:, 0:1024].rearrange("p (c m) -> p c m", m=64))
    nc.vector.memset(vaug[:, :, 64:65], 1.0)

    for g in range(2):
        susp = qpool.tile([128, 512], F32, tag="ps1")
        for jj in range(4):
            b = 4 * g + jj
            for cc in range(4):
                c = 4 * b + cc
                nc.tensor.matmul(
                    susp[:64, 65 * jj:65 * jj + 65],
                    wsb[:, c, :], vaug[:, c, :],
                    start=(jj == 0 and cc == 0), stop=(jj == 3 and cc == 3),
                    skip_group_check=True)
        sus_v = susp[:64, 0:260].rearrange("p (j e) -> p j e", j=4)
        suw_ap = sus_v[:, :, 64:65]
        suwm = spool.tile([64, 4, 1], F32, tag="suwm")
        nc.vector.tensor_scalar_max(suwm[:], suw_ap, 1e-6)
        rsw = spool.tile([64, 4, 1], F32, tag="rsw")
        nc.vector.reciprocal(out=rsw[:], in_=suwm[:])
        sge = spool.tile([64, 4, 1], F32, tag="sge")
        nc.scalar.activation(sge[:], suw_ap, AF.Exp, bias=0.0, scale=-1.0)
        sg = spool.tile([64, 4, 1], F32, tag="sg")
        nc.vector.tensor_scalar(sg[:], sge[:], -1.0, 1.0, op0=ALU.mult,
                                op1=ALU.add)
        mo = W["memold"][:, 4 * g:4 * g + 4, :]
        upd = s2pool.tile([64, 4, 64], F32, tag="wrtA")
        nc.vector.tensor_tensor(out=upd[:], in0=sus_v[:, :, 0:64],
                                in1=rsw[:].to_broadcast((64, 4, 64)),
                                op=ALU.mult)
        dlt = s2pool.tile([64, 4, 64], F32, tag="wrtB")
        nc.vector.tensor_tensor(out=dlt[:], in0=upd[:], in1=mo,
                                op=ALU.subtract)
        sdl = s2pool.tile([64, 4, 64], F32, tag="wrtA")
        nc.vector.tensor_tensor(out=sdl[:], in0=dlt[:],
                                in1=sg[:].to_broadcast((64, 4, 64)),
                                op=ALU.mult)
        nmo = s2pool.tile([64, 4, 64], F32, tag="wrtB")
        nc.vector.tensor_tensor(out=nmo[:], in0=sdl[:], in1=mo, op=ALU.add)
        nc.sync.dma_start(
            dram["memout"].ap()[:, 256 * g:256 * g + 256],
            nmo[:].rearrange("p j m -> p (j m)"))


# ======================= host side =======================================
_NC = {}


def _get_nc(gelu_sim=False):
    if gelu_sim not in _NC:
        _NC[gelu_sim] = _build(gelu_sim)
    return _NC[gelu_sim]


def _pack_shared(inp):
    """Pack weights shared by all cores -> dict of np arrays."""
    f = lambda x: np.asarray(x, dtype=np.float32)
    out = {}
    pc = _PAD_COLS

    def dup(a):  # duplicate [64, ...] over both partition groups -> [128,...]
        return np.concatenate([a, a], axis=0)

    wq = np.zeros((64, NB, 128), np.float32)
    wk = np.zeros((64, NB, 128), np.float32)
    wo = np.zeros((128, NB, 64), np.float32)
    for i in range(NB):
        wq[:, i, pc] = f(inp["wq"][i]) * 0.25   # fold 1/sqrt(HD)
        wk[:, i, pc] = f(inp["wk"][i])
        wo[pc, i, :] = f(inp["wo"][i])
    out["wq"] = dup(wq); out["wk"] = dup(wk); out["wo"] = wo
    out["wv"] = dup(f(inp["wv"]).transpose(1, 0, 2))       # [64, NB, 64]
    out["w1"] = dup(f(inp["ffn_w1"]).transpose(1, 0, 2))   # [64, NB, 256]
    b1 = f(inp["ffn_b1"])                                  # [NB, 256]
    out["b1"] = b1.reshape(NB, 2, 128).transpose(2, 0, 1).copy()
    out["w2"] = f(inp["ffn_w2"]).reshape(NB, 2, 128, 64).transpose(
        2, 0, 1, 3).copy()                                 # [128, NB, 2, 64]
    out["b2row"] = f(inp["ffn_b2"]).reshape(1, NB, 64)
    out["readq"] = dup(f(inp["read_q"]).transpose(1, 0, 2))  # [64, NB, 32]
    out["readout"] = f(inp["read_out"]).transpose(1, 0, 2)   # [64, NB, 64]

    ew = np.zeros((2, 14 * 128), np.float32)
    norms = []
    for i in range(NB):
        norms += [f(inp["attn_norm_w"][i]), f(inp["read_norm_w"][i]),
                  f(inp["ffn_norm_w"][i])]
    norms += [f(inp["out_norm_w"]), f(inp["write_norm_w"])]
    for idx, wn in enumerate(norms):
        ew[0, 128 * idx:128 * idx + 64] = wn
        ew[1, 128 * idx + 64:128 * idx + 128] = wn
    out["ew"] = ew

    p = np.arange(128)
    out["e2"] = (p[:, None] // 64 == np.arange(2)[None, :]).astype(np.float32)
    p64 = np.arange(64)
    out["e2a"] = (p64[:, None] // 32 == np.arange(2)[None, :]).astype(
        np.float32)
    out["e2at"] = out["e2a"].T.copy()
    kk = np.arange(128)
    out["ehw"] = (((kk[:, None] // 32) == (p[None, :] // 32)) &
                  ((kk[:, None] % 32) >= 16)).astype(np.float32) / 16.0
    out["negi"] = (-1e30 * np.eye(128)).astype(np.float32)
    out["triu"] = np.tril(np.ones((128, 128), np.float32), -1)
    out["ident"] = np.eye(128, dtype=np.float32)

    addr = f(inp["mem_addr"])                              # [M, A]
    an = addr / np.clip(np.sqrt((addr * addr).sum(-1, keepdims=True)),
                        1e-12, None)
    out["addrn"] = np.concatenate([an.T, an.T], axis=0)    # [64, 64]

    out["wqw"] = dup(f(inp["write_q"]))                    # [128, 32]
    out["wvw"] = dup(f(inp["write_v"]))                    # [128, 64]
    out["gw"] = dup(f(inp["gate_w"]))                      # [128, 1]
    out["gbneg"] = np.full((128, 1), -float(np.asarray(inp["gate_b"])[0]),
                           np.float32)
    return out


def _pack_core(x, mem, c):
    """Per-core tensors for core c."""
    xs = np.asarray(x[BL * c:BL * c + BL], np.float32)      # [8, 512, 64]
    ms = np.asarray(mem[BL * c:BL * c + BL], np.float32)    # [8, 64, 64]
    x2 = xs.reshape(2, 4, S, D).transpose(0, 3, 1, 2).reshape(128, FREE)
    ma = np.ones((64, BL, 65), np.float32)
    ma[:, :, 0:64] = ms.transpose(1, 0, 2)
    mo = ms.transpose(1, 0, 2).copy()
    return {"x2": np.ascontiguousarray(x2), "memaug": ma, "memold": mo}


LAST_RESULT = None


def kernel(**inputs):
    global LAST_RESULT
    nc = _get_nc()
    shared = _pack_shared(inputs)
    x = np.asarray(inputs["x"], np.float32)
    mem = np.asarray(inputs["memory_values"], np.float32)
    in_maps = []
    for c in range(NCORES):
        m = dict(shared)
        m.update(_pack_core(x, mem, c))
        in_maps.append(m)
    res = bass_utils.run_bass_kernel_spmd(
        nc, in_maps, list(range(NCORES)),
        trace=bool(os.environ.get("KTRACE")))
    LAST_RESULT = res
    xout = np.zeros((B, S, D), np.float32)
    memout = np.zeros((B, M, D), np.float32)
    for c in range(NCORES):
        r = res.results[c]
        x2 = r["xout2"].reshape(2, 64, 4, S).transpose(0, 2, 3, 1)
        xout[BL * c:BL * c + BL] = x2.reshape(BL, S, D)
        mm = r["memout"].reshape(64, BL, 64).transpose(1, 0, 2)
        memout[BL * c:BL * c + BL] = mm
    return xout, memout
